# revision 4
# baseline (speedup 1.0000x reference)
"""Trainium2 Bass kernel for the CgpHmmCell forward log-likelihood.

Computes loglik[b] = log-likelihood of each observation sequence under an
HMM with A = softmax(A_kernel, axis=1), Bm = softmax(B_kernel, axis=0),
initial mass on state 0 — matching the stabilized log-domain reference scan.

Strategy
--------
Data-parallel over batch: core c owns sequences 4c..4c+3.  Within a core the
T=4096 scan is broken into 64 time-chunks of L=64 steps; each (seq, chunk)
pair is an independent "lane" (256 lanes/core) run in lockstep in the LINEAR
domain (f <- (f @ A) * e_t, with emissions prescaled so the log-magnitude
drift stays centered).  HMM forward recursions are exponentially forgetting
(direction error decays ~10x/step for this operator family), so each chunk
burns in for H=16 steps from a uniform init before its segment; per-chunk
log-growth u_c = lnSum(end) - lnSum(burn-in end) is exact after burn-in and
sums to the sequence log-likelihood.  Chunk 0 is seeded exactly (delta on
state 0) at the step its segment starts.

Lane state is kept transposed ([state, lane]) so the per-step transition is
16 PE matmuls with A-tiles stationary (bf16) and the per-lane emission rows
are generated on-device from the one-hot inputs by PE transpose + a matmul
against Bm (so the full one-hot input is streamed from HBM, which is the
memory-bound part of the computation).
"""

import sys

sys.path.insert(0, "/opt/trn_rl_repo")

import numpy as np

import concourse.bass as bass
import concourse.tile as tile
from concourse import bacc, mybir
from concourse.bass import ts
from concourse.bass_utils import run_bass_kernel_spmd
from concourse.masks import make_identity

# problem shapes (hardcoded per contract)
B, T, S, E = 32, 4096, 512, 128
NCORES = 8
BPC = B // NCORES          # sequences per core = 4
L = 64                     # chunk length
H = 16                     # burn-in steps
NCHUNK = T // L            # 64 chunks per sequence
LANES = BPC * NCHUNK       # 256 lanes per core
STEPS = H + L              # 80 lockstep steps
W = 8                      # one-hot DMA block (steps per DMA batch)
LNS = float(np.log(128.0) - 0.05)   # emission prescale (log)
SJ = S // 128              # 4 state tiles
FP32 = mybir.dt.float32
BF16 = mybir.dt.bfloat16


def _emit_e_stage_load(nc, xpool, x_ap, k0):
    """DMA one-hot rows for steps k0..k0+W-1 (both lane groups).

    Lane (b_local, c) at loop step k consumes emission time t = c*L + (k - H),
    i.e. row x[b, t] with t viewed as (c, l): l = k - H (shifting into c-1 for
    l < 0).  c=0 lanes have t < 0 during burn-in (k < H): they read x[b, 0]
    (clamped garbage; their state is re-seeded exactly at k == H).
    Returns [xg0, xg1] staging tiles of shape [128, W, 128] (partition=lane).
    """
    tiles = []
    for g in range(2):
        xg = xpool.tile([128, W, E], FP32, tag=f"x{g}")
        for bb in range(2):
            b = 2 * g + bb
            xr = x_ap[b].rearrange("(c l) e -> c l e", l=L)  # [64, 64, 128]
            if k0 >= H:
                nc.sync.dma_start(
                    out=xg[64 * bb : 64 * bb + 64],
                    in_=xr[:, k0 - H : k0 - H + W, :],
                )
            else:
                # c >= 1 lanes borrow from chunk c-1: t = c*L + (k-H)
                #   = (c-1)*L + (L + k - H)
                nc.sync.dma_start(
                    out=xg[64 * bb + 1 : 64 * bb + 64],
                    in_=xr[0 : NCHUNK - 1, L + k0 - H : L + k0 - H + W, :],
                )
                # c == 0 lane: clamp to t=0 row, broadcast across the W steps
                src = x_ap[b, 0:1, :]
                bcast = bass.AP(
                    tensor=src.tensor,
                    offset=src.offset,
                    ap=[[0, 1], [0, W], [1, E]],
                )
                nc.sync.dma_start(
                    out=xg[64 * bb : 64 * bb + 1, :, :],
                    in_=bcast,
                )
        tiles.append(xg)
    return tiles


def build_program():
    nc = bacc.Bacc("TRN2", debug=False, num_devices=NCORES)

    x = nc.dram_tensor("x", [BPC, T, E], FP32, kind="ExternalInput")
    ak = nc.dram_tensor("ak", [S, S], FP32, kind="ExternalInput")
    bk = nc.dram_tensor("bk", [E, S], FP32, kind="ExternalInput")
    out = nc.dram_tensor("out", [1, BPC], FP32, kind="ExternalOutput")

    with tile.TileContext(nc) as tc:
        with (
            tc.tile_pool(name="singles", bufs=1) as singles,
            tc.tile_pool(name="prep", bufs=2) as prep,
            tc.tile_pool(name="xpool", bufs=2) as xpool,
            tc.tile_pool(name="xtsb", bufs=2) as xtsb,
            tc.tile_pool(name="esb", bufs=3) as esb,
            tc.tile_pool(name="phip", bufs=2) as phip,
            tc.tile_pool(name="main_ps", bufs=1, space="PSUM") as main_psp,
            tc.tile_pool(name="e_ps", bufs=2, space="PSUM") as e_psp,
            tc.tile_pool(name="misc_ps", bufs=2, space="PSUM") as misc_psp,
        ):
            # ---------------- one-time prep ----------------
            ident = singles.tile([128, 128], FP32)
            make_identity(nc, ident)

            # A = softmax(ak, axis=1), rows (s_in) on partitions -> bf16 tiles
            a_sb = prep.tile([128, SJ, S], FP32, tag="a_stage")
            nc.sync.dma_start(out=a_sb, in_=ak.ap().rearrange("(ko ki) m -> ki ko m", ki=128))
            A_bf = singles.tile([128, SJ, S], BF16)
            for ko in range(SJ):
                expa = prep.tile([128, S], FP32, tag="expa")
                zs = prep.tile([128, 1], FP32, tag="zs")
                nc.scalar.activation(
                    out=expa, in_=a_sb[:, ko, :],
                    func=mybir.ActivationFunctionType.Exp, accum_out=zs,
                )
                rec = prep.tile([128, 1], FP32, tag="rec")
                nc.vector.reciprocal(out=rec, in_=zs)
                nc.vector.tensor_scalar_mul(out=A_bf[:, ko, :], in0=expa, scalar1=rec)

            # Bm = softmax(bk, axis=0) * exp(LNS), emissions (e) on partitions
            b_sb = prep.tile([E, S], FP32, tag="b_stage")
            nc.sync.dma_start(out=b_sb, in_=bk.ap())
            expb = prep.tile([E, S], FP32, tag="expb")
            nc.scalar.activation(out=expb, in_=b_sb, func=mybir.ActivationFunctionType.Exp)
            ones_col = singles.tile([E, 1], FP32)
            nc.vector.memset(ones_col, 1.0)
            zb_ps = misc_psp.tile([1, S], FP32, tag="xt_ps")
            nc.tensor.matmul(zb_ps, lhsT=ones_col, rhs=expb, start=True, stop=True)
            recb = prep.tile([1, S], FP32, tag="recb")
            nc.vector.reciprocal(out=recb, in_=zb_ps)
            nc.vector.tensor_scalar_mul(out=recb, in0=recb, scalar1=float(np.exp(LNS)))
            ones_row = singles.tile([1, 128], FP32)
            nc.vector.memset(ones_row, 1.0)
            bc_ps = misc_psp.tile([128, S], FP32, tag="xt_ps")
            nc.tensor.matmul(bc_ps, lhsT=ones_row, rhs=recb, start=True, stop=True)
            Bm_bf = singles.tile([E, S], BF16)
            nc.vector.tensor_tensor(
                out=Bm_bf, in0=expb, in1=bc_ps, op=mybir.AluOpType.mult
            )

            ones_s = singles.tile([128, 1], BF16)
            nc.vector.memset(ones_s, 1.0)
            lnS_start = singles.tile([1, LANES], FP32)
            lnS_end = singles.tile([1, LANES], FP32)

            # ---------------- E-generation pipeline ----------------
            x_ap = x.ap()
            xg_cur = {}

            def e_transpose(k):
                """PE-transpose one-hot rows for step k, copy to bf16 SBUF."""
                if k % W == 0:
                    xg_cur[0], xg_cur[1] = _emit_e_stage_load(nc, xpool, x_ap, k)
                xt_ps = misc_psp.tile([128, 2, 128], FP32, tag="xt_ps")
                xt_sb = xtsb.tile([128, 2 * 128], BF16, tag="xt_sb")
                for g in range(2):
                    nc.tensor.transpose(xt_ps[:, g, :], xg_cur[g][:, k % W, :], ident)
                for g in range(2):
                    nc.vector.tensor_copy(out=xt_sb[:, ts(g, 128)], in_=xt_ps[:, g, :])
                return xt_sb

            def e_matmul(k, xt_sb):
                """Emission rows for step k: e[s, lane] = Bm[obs, s] (scaled)."""
                e_ps = e_psp.tile([128, SJ, LANES], FP32, tag="e_ps")
                for j in range(SJ):
                    nc.tensor.matmul(
                        e_ps[:, j, :], lhsT=Bm_bf[:, ts(j, 128)], rhs=xt_sb,
                        start=True, stop=True,
                    )
                e_sb = esb.tile([128, SJ, LANES], BF16, tag="e_sb")
                for j in range(SJ):
                    nc.scalar.copy(out=e_sb[:, j, :], in_=e_ps[:, j, :])
                return e_sb

            # ---------------- main lockstep scan ----------------
            # software pipeline: transpose k+1 ahead, e-matmul k ahead
            xt_q = [e_transpose(0), e_transpose(1)]
            e_q = [e_matmul(0, xt_q[0])]

            phi = None
            for k in range(STEPS):
                # advance E pipeline (keeps PE fed while DVE does multiplies)
                if k + 2 < STEPS:
                    xt_q.append(e_transpose(k + 2))
                if k + 1 < STEPS:
                    e_q.append(e_matmul(k + 1, xt_q[k + 1]))
                e_sb = e_q[k]

                phi_new = phip.tile([128, SJ, LANES], BF16, tag="phi")
                if k == 0:
                    # phi = 1 * e
                    for j in range(SJ):
                        nc.vector.tensor_copy(out=phi_new[:, j, :], in_=e_sb[:, j, :])
                else:
                    main_ps = main_psp.tile([128, SJ, LANES], FP32, tag="main_ps")
                    for j in range(SJ):
                        for i in range(SJ):
                            nc.tensor.matmul(
                                main_ps[:, j, :],
                                lhsT=A_bf[:, i, ts(j, 128)],
                                rhs=phi[:, i, :],
                                start=(i == 0),
                                stop=(i == SJ - 1),
                            )
                    for j in range(SJ):
                        nc.vector.tensor_tensor(
                            out=phi_new[:, j, :], in0=main_ps[:, j, :],
                            in1=e_sb[:, j, :], op=mybir.AluOpType.mult,
                        )

                if k == H:
                    # exact re-seed of chunk-0 lanes (cols 0,64,128,192):
                    # phi[:, lane] = delta(state 0) * e_t0[:, lane]
                    for j in range(SJ):
                        nc.vector.memset(
                            phi_new[:, j, :].rearrange("p (b c) -> p b c", c=L)[:, :, 0],
                            0.0,
                        )
                    nc.vector.tensor_copy(
                        out=phi_new[0:1, 0, :].rearrange("p (b c) -> p b c", c=L)[:, :, 0],
                        in_=e_sb[0:1, 0, :].rearrange("p (b c) -> p b c", c=L)[:, :, 0],
                    )

                if k in (H - 1, STEPS - 1):
                    ck_ps = misc_psp.tile([1, LANES], FP32, tag="xt_ps")
                    for i in range(SJ):
                        nc.tensor.matmul(
                            ck_ps, lhsT=ones_s, rhs=phi_new[:, i, :],
                            start=(i == 0), stop=(i == SJ - 1),
                        )
                    tgt = lnS_start if k == H - 1 else lnS_end
                    nc.scalar.activation(
                        out=tgt, in_=ck_ps, func=mybir.ActivationFunctionType.Ln
                    )

                phi = phi_new

            # ---------------- combine ----------------
            u = prep.tile([1, LANES], FP32, tag="u")
            nc.vector.tensor_tensor(
                out=u, in0=lnS_end, in1=lnS_start, op=mybir.AluOpType.subtract
            )
            # chunk-0 lanes have no burn-in reference: u = lnS_end
            nc.vector.tensor_add(
                out=u.rearrange("p (b c) -> p b c", c=L)[:, :, 0],
                in0=u.rearrange("p (b c) -> p b c", c=L)[:, :, 0],
                in1=lnS_start.rearrange("p (b c) -> p b c", c=L)[:, :, 0],
            )
            ub = prep.tile([1, BPC], FP32, tag="ub")
            nc.vector.tensor_reduce(
                out=ub,
                in_=u.rearrange("p (b c) -> p b c", c=L),
                axis=mybir.AxisListType.X,
                op=mybir.AluOpType.add,
            )
            # undo the per-step prescale: every one of the T emissions was
            # multiplied by exp(LNS)
            nc.vector.tensor_scalar_add(out=ub, in0=ub, scalar1=float(-T * LNS))
            nc.sync.dma_start(out=out.ap(), in_=ub)

    nc.compile()
    return nc


_NC = None


def _get_nc():
    global _NC
    if _NC is None:
        _NC = build_program()
    return _NC


def kernel(inputs: np.ndarray, A_kernel: np.ndarray, B_kernel: np.ndarray) -> np.ndarray:
    nc = _get_nc()
    ak = np.ascontiguousarray(A_kernel, dtype=np.float32)
    bk = np.ascontiguousarray(B_kernel, dtype=np.float32)
    in_maps = [
        {
            "x": np.ascontiguousarray(inputs[BPC * c : BPC * (c + 1)], dtype=np.float32),
            "ak": ak,
            "bk": bk,
        }
        for c in range(NCORES)
    ]
    res = run_bass_kernel_spmd(nc, in_maps, core_ids=list(range(NCORES)))
    out = np.concatenate([res.results[c]["out"].reshape(BPC) for c in range(NCORES)])
    return out.reshape(B, 1).astype(np.float32)
